# revision 19
# baseline (speedup 1.0000x reference)
"""Trainium2 Bass kernel for 5-layer GAT (nn_BERT_GAT).

Self-contained: kernel(**inputs) takes FULL inputs, shards nodes across 8
NeuronCores (graph parallel, dst-sharded), runs one SPMD NEFF that executes
all 5 GAT layers (feat matmul -> AllGather of bf16 feature table ->
dma_gather edge gathers -> pattern-matmul segment softmax/aggregation),
and returns (x [50000,200] f32, alpha [300000,1] f32).

Host-side preprocessing builds per-core block geometry:
  - 50 blocks/core x 125 consecutive dst nodes each
  - each block has 1280 edge slots (10 chunks of 128): slots 0..639 for
    edges with src<25000 ("lo"), 640..1279 for src>=25000 ("hi") so that
    int16 dma_gather indices stay in range; trailing -1 pads are skipped
    by the DMA.
  - pattern matrices PT [slot,dst] / PN [dst,slot] (bf16 0/1) turn segment
    sum / per-dst broadcast into tensor-engine matmuls.

Feature-table row layout (bf16), layers 0..3 (cols=896):
  [f0(200) | 1 | f1(200) | 1 | f2(200) | 1 | f3(200) | 1 | al_s(4) | al_d(4) | 0pad]
layer 4 (cols=256):
  [f(200) | 1 | al_s | al_d | 0pad]
The per-head "1" column makes the segment-sum s ride along in the same
aggregation matmul (column 200 of each head's N=201 matmul).
"""

import os
import numpy as np
import ml_dtypes

BF16 = ml_dtypes.bfloat16

N = 50000
E = 250000
H = 4
C = 200
NEG_SLOPE = 0.2
NCORES = 8
NSHARD = N // NCORES            # 6250
DST_PER_BLOCK = 125
B = NSHARD // DST_PER_BLOCK     # 50
M_HALF = 5                      # 128-chunks per half
KSLOT = 2 * M_HALF * 128        # 1280
HALF = N // 2                   # 25000
COLS = 896                      # layers 0-3 table row (bf16) = 1792B
COLS4 = 256                     # layer 4 table row = 512B
NT = (NSHARD + 127) // 128      # 49 node tiles
NROWPAD = 7168                  # x_rows padded rows (7*1024)
NG = NROWPAD // 1024            # 7 transpose groups
HEADS = [4, 4, 4, 4, 1]
FIN = [200, 800, 800, 800, 800]
ALS_OFF = {896: 804, 256: 201}  # al_s col offset per table layout
ALD_OFF = {896: 808, 256: 202}
XCOLS0 = 256                    # padded layer-0 input rows (Fin=200 -> 256)

LAST_RESULT = None              # test.py introspection


# ----------------------------------------------------------------------------
# Host preprocessing
# ----------------------------------------------------------------------------

def build_plan(edge_index):
    """Block geometry + gather indices + patterns + alpha bookkeeping."""
    src = np.concatenate([np.asarray(edge_index[0], np.int64),
                          np.arange(N, dtype=np.int64)])
    dst = np.concatenate([np.asarray(edge_index[1], np.int64),
                          np.arange(N, dtype=np.int64)])
    Eprime = src.shape[0]
    order = np.argsort(dst, kind="stable")
    s_src = src[order]
    s_dst = dst[order]
    dst_starts = np.searchsorted(s_dst, np.arange(N + 1))

    alpha_core = np.empty(Eprime, np.int32)
    alpha_block = np.empty(Eprime, np.int32)
    alpha_slot = np.empty(Eprime, np.int32)

    cores = []
    for k in range(NCORES):
        IDX = np.full((B, 128, KSLOT // 16), -1, np.int16)
        CNT = np.zeros(128, np.int32)
        PT = np.zeros((B, 128, KSLOT), BF16)   # [b, slot%128, c*128 -> slot, dst]
        PN = np.zeros((B, 128, KSLOT), BF16)
        for b in range(B):
            d0 = k * NSHARD + b * DST_PER_BLOCK
            e0 = dst_starts[d0]
            e1 = dst_starts[d0 + DST_PER_BLOCK]
            eidx = order[e0:e1]
            bsrc = s_src[e0:e1]
            bdst = s_dst[e0:e1]
            lo_pos = np.nonzero(bsrc < HALF)[0]
            hi_pos = np.nonzero(bsrc >= HALF)[0]
            n_lo, n_hi = len(lo_pos), len(hi_pos)
            if n_lo > M_HALF * 128 or n_hi > M_HALF * 128:
                raise RuntimeError(f"block overflow core {k} blk {b}: {n_lo},{n_hi}")
            # slot -> edge position
            slots = np.full(KSLOT, -1, np.int64)
            slots[:n_lo] = lo_pos
            slots[M_HALF * 128:M_HALF * 128 + n_hi] = hi_pos

            idx = np.full(KSLOT, -1, np.int16)
            idx[:n_lo] = bsrc[lo_pos].astype(np.int16)
            idx[M_HALF * 128:M_HALF * 128 + n_hi] = (bsrc[hi_pos] - HALF).astype(np.int16)
            nv_lo, nv_hi = n_lo, n_hi
            if nv_lo == 0:
                idx[0] = 0
                nv_lo = 1
            if nv_hi == 0:
                idx[M_HALF * 128] = 0
                nv_hi = 1
            # dma_gather idx layout: value i at [i%16 (replicated %16), i//16]
            # per-call arrays are slot-local (0..639) for each half
            lo = idx[:M_HALF * 128].reshape(-1, 16).T          # [16, 40]
            hi = idx[M_HALF * 128:].reshape(-1, 16).T
            IDX[b, :, :KSLOT // 32] = np.tile(lo, (8, 1))
            IDX[b, :, KSLOT // 32:] = np.tile(hi, (8, 1))
            CNT[2 * b] = nv_lo
            CNT[2 * b + 1] = nv_hi

            val = np.nonzero(slots >= 0)[0]
            dstloc = (bdst[slots[val]] - d0).astype(np.int64)
            p_ = val % 128
            c_ = val // 128
            PT[b, p_, c_ * 128 + dstloc] = BF16(1.0)
            PN[b, dstloc, c_ * 128 + p_] = BF16(1.0)

            ge = eidx[slots[val]]
            alpha_core[ge] = k
            alpha_block[ge] = b
            alpha_slot[ge] = val
        cores.append(dict(IDX=IDX, CNT=CNT, PT=PT, PN=PN))
    return cores, (alpha_core, alpha_block, alpha_slot)


def build_wext(W, a_s, a_d, h, fin_pad, cols):
    """W_ext [fin_pad, cols] bf16: rows follow the padded x layout of the
    PREVIOUS layer's table (zero rows at ones/pad positions); cols follow the
    table layout of THIS layer (zeros at ones/pad cols)."""
    hC = h * C
    Wx = np.zeros((fin_pad, cols), np.float32)
    fin = W.shape[0]
    # map W rows -> padded x rows
    if fin == 200:
        rowmap = np.arange(200)
    else:
        # previous layer had 4 heads with the 201-stride layout
        rowmap = np.concatenate([np.arange(hh * 201, hh * 201 + 200)
                                 for hh in range(4)])
    als = ALS_OFF[cols]
    ald = ALD_OFF[cols]
    for hh in range(h):
        Wx[rowmap, hh * 201:hh * 201 + 200] = W[:, hh * C:(hh + 1) * C]
        Wx[rowmap, als + hh] = W[:, hh * C:(hh + 1) * C] @ a_s[hh]
        Wx[rowmap, ald + hh] = W[:, hh * C:(hh + 1) * C] @ a_d[hh]
    return np.asarray(Wx, BF16)


# ----------------------------------------------------------------------------
# Bass program
# ----------------------------------------------------------------------------

def build_program(nlayers=5, gelu_mode=None):
    if gelu_mode is None:
        gelu_mode = os.environ.get("GAT_GELU", "native")
    import concourse.bass as bass
    import concourse.bacc as bacc
    import concourse.tile as tile
    from concourse import mybir

    FP32 = mybir.dt.float32
    BF = mybir.dt.bfloat16
    I16 = mybir.dt.int16
    I32 = mybir.dt.int32
    AF = mybir.ActivationFunctionType
    ALU = mybir.AluOpType

    nc = bacc.Bacc("TRN2", target_bir_lowering=False, debug=False,
                   num_devices=NCORES)

    # ---- I/O ----
    x0T = nc.dram_tensor("x0T", [XCOLS0, NROWPAD], BF, kind="ExternalInput")
    wext = []
    for l in range(5):
        fin_pad = XCOLS0 if l == 0 else COLS
        cols = COLS4 if l == 4 else COLS
        wext.append(nc.dram_tensor(f"wext{l}", [fin_pad, cols], BF,
                                   kind="ExternalInput"))
    PT_d = nc.dram_tensor("PT", [B, 128, KSLOT], BF, kind="ExternalInput")
    PN_d = nc.dram_tensor("PN", [B, 128, KSLOT], BF, kind="ExternalInput")
    IDX_d = nc.dram_tensor("IDX", [B, 128, KSLOT // 16], I16, kind="ExternalInput")
    CNT_d = nc.dram_tensor("CNT", [1, 128], I32, kind="ExternalInput")

    out4 = nc.dram_tensor("out4", [NSHARD, 200], FP32, kind="ExternalOutput")
    alpha_o = nc.dram_tensor("alpha", [B, KSLOT], FP32, kind="ExternalOutput")
    # debug output: bf16 x_rows after layer `nlayers-1` (only used when
    # nlayers < 5 during bringup)
    dbg = None
    if nlayers < 5:
        dbg = nc.dram_tensor("dbg", [NSHARD, COLS], FP32, kind="ExternalOutput")

    with tile.TileContext(nc) as tc:
        with (
            tc.tile_pool(name="dram", bufs=1, space="DRAM") as dram,
            tc.tile_pool(name="persist", bufs=1) as persist,
        ):
            # DRAM working tensors
            bounce = [dram.tile([NSHARD, COLS], BF, name=f"bounce{i}")
                      for i in range(2)]
            table = [dram.tile([N, COLS], BF, addr_space="Shared",
                               name=f"table{i}") for i in range(4)]
            bounce4 = dram.tile([NSHARD, COLS4], BF, name="bounce4")
            table4 = dram.tile([N, COLS4], BF, addr_space="Shared", name="table4")
            x_rows = [dram.tile([NROWPAD, COLS], BF, name=f"xrows{i}")
                      for i in range(2)]
            ald_dram = dram.tile([NT * 128, 4], BF, name="ald_dram")

            # resident SBUF: counts
            cnt_sb = persist.tile([1, 128], I32, name="cnt_sb")
            nc.sync.dma_start(out=cnt_sb[:], in_=CNT_d[:])

            # G gather buffers: manually double-buffered, memset once so that
            # slots skipped by dma_gather (trailing -1 idx) always hold finite
            # stale data (pattern columns are 0 there; 0*NaN would poison PSUM)
            g_bufs = [persist.tile([128, 2 * M_HALF, COLS], BF, name=f"gbuf{i}")
                      for i in range(2)]
            g4_bufs = [persist.tile([128, 2 * M_HALF, COLS4], BF, name=f"g4buf{i}")
                       for i in range(2)]
            for gb in g_bufs + g4_bufs:
                nc.vector.memset(gb[:], 0.0)

            # zero x_rows pad region (rows 6250..7168) once; cols are fully
            # written each layer. Also zero a 128-part staging tile for it.
            zpad = persist.tile([128, COLS], BF, name="zpad")
            nc.vector.memset(zpad[:], 0.0)
            for r0 in range(NSHARD, NROWPAD, 128):
                rows = min(128, NROWPAD - r0)
                for xr in x_rows:
                    nc.sync.dma_start(out=xr[r0:r0 + rows, :], in_=zpad[:rows, :])

            def feat_phase(l):
                """x -> table shard (bounce) + al_d staging."""
                h = HEADS[l]
                cols = COLS4 if l == 4 else COLS
                nch = 2 if l == 0 else 7
                bnc = bounce4 if l == 4 else bounce[l % 2]
                xin = x_rows[(l - 1) % 2] if l > 0 else None
                with (
                    tc.tile_pool(name=f"fpsum{l}", bufs=2, space="PSUM") as fpsum,
                    tc.tile_pool(name=f"fsb{l}", bufs=2) as fsb,
                    tc.tile_pool(name=f"fxt{l}", bufs=2) as fxt,
                    tc.tile_pool(name=f"fw{l}", bufs=1) as fw,
                ):
                    w_sb = fw.tile([128, nch, cols], BF, name=f"w_sb{l}")
                    nc.sync.dma_start(
                        out=w_sb[:],
                        in_=wext[l].rearrange("(c p) n -> p c n", p=128))
                    al_stage = fw.tile([128, NT, 4], BF, name=f"al_stage{l}")
                    for g in range(NG):
                        xt = fxt.tile([128, nch, 1024], BF, tag="xt")
                        for cc in range(nch):
                            if l == 0:
                                nc.sync.dma_start(
                                    out=xt[:, cc, :],
                                    in_=x0T[cc * 128:(cc + 1) * 128,
                                            g * 1024:(g + 1) * 1024])
                            else:
                                nc.sync.dma_start_transpose(
                                    out=xt[:, cc, :],
                                    in_=xin[g * 1024:(g + 1) * 1024,
                                            cc * 128:(cc + 1) * 128])
                        for ts in range(8):
                            t = g * 8 + ts
                            if t >= NT:
                                break
                            pf = fpsum.tile([128, cols], FP32, tag="pf")
                            for cc in range(nch):
                                for n0 in range(0, cols, 512):
                                    n1 = min(n0 + 512, cols)
                                    nc.tensor.matmul(
                                        out=pf[:, n0:n1],
                                        lhsT=xt[:, cc, ts * 128:(ts + 1) * 128],
                                        rhs=w_sb[:, cc, n0:n1],
                                        start=(cc == 0), stop=(cc == nch - 1))
                            sbf = fsb.tile([128, cols], BF, tag="sbf")
                            nc.scalar.activation(sbf[:], pf[:], AF.Copy)
                            # ones columns (h cols, stride 201)
                            nc.vector.memset(
                                sbf[:, 200:200 + 201 * (h - 1) + 1:201], 1.0)
                            # stage al_d (bf16)
                            nc.vector.tensor_copy(
                                out=al_stage[:, t, :h],
                                in_=pf[:, ALD_OFF[cols]:ALD_OFF[cols] + h])
                            rows = min(128, NSHARD - t * 128)
                            nc.sync.dma_start(out=bnc[t * 128:t * 128 + rows, :],
                                              in_=sbf[:rows, :])
                    # al_d staging -> dram (node = t*128+p)
                    nc.sync.dma_start(
                        out=ald_dram[:].rearrange("(t p) c -> p t c", p=128),
                        in_=al_stage[:])

            def allgather(l):
                cols = COLS4 if l == 4 else COLS
                bnc = bounce4 if l == 4 else bounce[l % 2]
                tbl = table4 if l == 4 else table[l]
                nc.gpsimd.collective_compute(
                    "AllGather", ALU.bypass,
                    replica_groups=[list(range(NCORES))],
                    ins=[bnc[:].opt()],
                    outs=[tbl[:].opt()],
                )

            def emit_gelu_norm(pool, out_ap, psum_ap, rs_ap, w):
                """out = gelu_tanh(psum * rs); rs per-partition [128,1]."""
                if gelu_mode == "native":
                    nc.scalar.activation(out_ap, psum_ap, AF.Gelu_apprx_tanh,
                                         scale=rs_ap)
                    return
                y = pool.tile([128, w], FP32, tag="gy", name="gy")
                nc.scalar.activation(y[:], psum_ap, AF.Copy, scale=rs_ap)
                u = pool.tile([128, w], FP32, tag="gu", name="gu")
                nc.vector.tensor_mul(out=u[:], in0=y[:], in1=y[:])
                nc.vector.tensor_scalar(out=u[:], in0=u[:], scalar1=0.044715,
                                        scalar2=1.0, op0=ALU.mult, op1=ALU.add)
                nc.vector.tensor_mul(out=u[:], in0=u[:], in1=y[:])
                t = pool.tile([128, w], FP32, tag="gt", name="gt")
                nc.scalar.activation(t[:], u[:], AF.Tanh,
                                     scale=0.7978845608028654)
                nc.vector.scalar_tensor_tensor(out=t[:], in0=t[:], scalar=1.0,
                                               in1=y[:], op0=ALU.add,
                                               op1=ALU.mult)
                nc.vector.tensor_scalar_mul(out_ap, t[:], 0.5)

            def edge_phase(l):
                h = HEADS[l]
                cols = COLS4 if l == 4 else COLS
                tbl = table4 if l == 4 else table[l]
                xout = x_rows[l % 2]
                als_o = ALS_OFF[cols]
                NREG = 4
                cnt_regs = [nc.gpsimd.alloc_register(f"cntreg{l}_{i}")
                            for i in range(2 * NREG)]
                with (
                    tc.tile_pool(name=f"eg{l}", bufs=2) as eg,
                    tc.tile_pool(name=f"esb{l}", bufs=2) as esb,
                    tc.tile_pool(name=f"epsum{l}", bufs=2, space="PSUM") as eps,
                    tc.tile_pool(name=f"eout{l}", bufs=2) as eout,
                ):
                    for b in range(B):
                        pt_sb = esb.tile([128, M_HALF * 2, 128], BF, tag="pt")
                        nc.sync.dma_start(out=pt_sb[:],
                                          in_=PT_d[b].rearrange("p (c q) -> p c q", q=128))
                        pn_sb = esb.tile([128, M_HALF * 2, 128], BF, tag="pn")
                        nc.sync.dma_start(out=pn_sb[:],
                                          in_=PN_d[b].rearrange("p (c q) -> p c q", q=128))
                        idx_sb = esb.tile([128, KSLOT // 16], I16, tag="idx")
                        nc.sync.dma_start(out=idx_sb[:], in_=IDX_d[b])
                        ald_sb = esb.tile([128, 4], BF, tag="ald")
                        nc.sync.dma_start(out=ald_sb[:],
                                          in_=ald_dram[b * DST_PER_BLOCK:
                                                       b * DST_PER_BLOCK + 128, :])

                        g_sb = (g4_bufs if l == 4 else g_bufs)[b % 2]
                        nlo = cnt_regs[(b % NREG) * 2]
                        nhi = cnt_regs[(b % NREG) * 2 + 1]
                        nc.gpsimd.reg_load(nlo, cnt_sb[0:1, 2 * b:2 * b + 1])
                        nc.gpsimd.reg_load(nhi, cnt_sb[0:1, 2 * b + 1:2 * b + 2])
                        nc.gpsimd.dma_gather(
                            out_ap=g_sb[:, 0:M_HALF, :],
                            in_ap=tbl[0:HALF, :],
                            idxs_ap=idx_sb[:, 0:KSLOT // 32],
                            num_idxs=M_HALF * 128,
                            num_idxs_reg=nlo,
                            elem_size=cols)
                        nc.gpsimd.dma_gather(
                            out_ap=g_sb[:, M_HALF:2 * M_HALF, :],
                            in_ap=tbl[HALF:N, :],
                            idxs_ap=idx_sb[:, KSLOT // 32:],
                            num_idxs=M_HALF * 128,
                            num_idxs_reg=nhi,
                            elem_size=cols)

                        # e_d expand: [slot, h] per chunk
                        ed_ps = eps.tile([128, 2 * M_HALF, 4], FP32, tag="ed")
                        for cc in range(2 * M_HALF):
                            nc.tensor.matmul(out=ed_ps[:, cc, :h],
                                             lhsT=pn_sb[:, cc, :],
                                             rhs=ald_sb[:, :h],
                                             start=True, stop=True)
                        # e = lrelu(al_s + e_d) ; p = exp(e)
                        alsf_sb = esb.tile([128, 2 * M_HALF, 4], FP32, tag="alsf")
                        nc.vector.tensor_copy(out=alsf_sb[:, :, :h],
                                              in_=g_sb[:, :, als_o:als_o + h])
                        e_sb = esb.tile([128, 2 * M_HALF, 4], FP32, tag="e")
                        nc.vector.tensor_add(out=e_sb[:, :, :h],
                                             in0=ed_ps[:, :, :h],
                                             in1=alsf_sb[:, :, :h])
                        e2_sb = esb.tile([128, 2 * M_HALF, 4], FP32, tag="e2")
                        nc.vector.tensor_scalar_mul(e2_sb[:, :, :h],
                                                    e_sb[:, :, :h], NEG_SLOPE)
                        nc.vector.tensor_max(out=e_sb[:, :, :h],
                                             in0=e_sb[:, :, :h],
                                             in1=e2_sb[:, :, :h])
                        p_sb = esb.tile([128, 2 * M_HALF, 4], FP32, tag="p")
                        nc.scalar.activation(p_sb[:, :, :h], e_sb[:, :, :h], AF.Exp)
                        pbf_sb = esb.tile([128, 2 * M_HALF, 4], BF, tag="pbf")
                        nc.vector.tensor_copy(out=pbf_sb[:, :, :h],
                                              in_=p_sb[:, :, :h])

                        # S_T' = PT * p  (bf16) [slot, head, dst]
                        st_sb = esb.tile([128, 2 * M_HALF, 4, 128], BF, tag="st")
                        for cc in range(2 * M_HALF):
                            nc.vector.tensor_tensor(
                                out=st_sb[:, cc, :h, :],
                                in0=pt_sb[:, cc:cc + 1, :].broadcast_to([128, h, 128]),
                                in1=pbf_sb[:, cc, :h, None].broadcast_to([128, h, 128]),
                                op=ALU.mult)

                        # aggregation matmuls: out'[dst, 201] per head
                        op_ps = []
                        for pair in range((h + 1) // 2):
                            op_ps.append(eps.tile([128, 2, 201], FP32, tag="op",
                                                  name=f"op{pair}"))
                        for hh in range(h):
                            for cc in range(2 * M_HALF):
                                nc.tensor.matmul(
                                    out=op_ps[hh // 2][:, hh % 2, :],
                                    lhsT=st_sb[:, cc, hh, :],
                                    rhs=g_sb[:, cc, hh * 201:hh * 201 + 201],
                                    start=(cc == 0), stop=(cc == 2 * M_HALF - 1))

                        # s -> 1/(s+eps)
                        s_sb = esb.tile([128, 4], FP32, tag="s")
                        for hh in range(h):
                            nc.vector.tensor_copy(
                                out=s_sb[:, hh:hh + 1],
                                in_=op_ps[hh // 2][:, hh % 2, 200:201])
                        rs_sb = esb.tile([128, 4], FP32, tag="rs")
                        nc.vector.tensor_scalar_add(rs_sb[:, :h], s_sb[:, :h], 1e-16)
                        nc.vector.reciprocal(rs_sb[:, :h], rs_sb[:, :h])

                        # normalize + gelu -> x_rows / out4
                        if l < 4:
                            xa = eout.tile([128, COLS], BF, tag="xa")
                            nc.vector.memset(xa[:], 0.0)
                            for hh in range(h):
                                emit_gelu_norm(
                                    eout,
                                    xa[:, hh * 201:hh * 201 + 200],
                                    op_ps[hh // 2][:, hh % 2, 0:200],
                                    rs_sb[:, hh:hh + 1], 200)
                            if l == nlayers - 1 and dbg is not None:
                                xf = eout.tile([128, COLS], FP32, tag="xf")
                                nc.vector.tensor_copy(out=xf[:], in_=xa[:])
                                nc.sync.dma_start(
                                    out=dbg[b * DST_PER_BLOCK:
                                            b * DST_PER_BLOCK + DST_PER_BLOCK, :],
                                    in_=xf[:DST_PER_BLOCK, :])
                            nc.sync.dma_start(
                                out=xout[b * DST_PER_BLOCK:
                                         b * DST_PER_BLOCK + DST_PER_BLOCK, :],
                                in_=xa[:DST_PER_BLOCK, :])
                        else:
                            xo = eout.tile([128, 200], FP32, tag="xo")
                            emit_gelu_norm(eout, xo[:], op_ps[0][:, 0, 0:200],
                                           rs_sb[:, 0:1], 200)
                            nc.sync.dma_start(
                                out=out4[b * DST_PER_BLOCK:
                                         b * DST_PER_BLOCK + DST_PER_BLOCK, :],
                                in_=xo[:DST_PER_BLOCK, :])
                            # alpha = p * s_exp (PN cast to fp32 to keep the
                            # per-dst 1/s exact in fp32)
                            pnf_sb = eout.tile([128, 2 * M_HALF, 128], FP32,
                                               tag="pnf")
                            nc.vector.tensor_copy(out=pnf_sb[:], in_=pn_sb[:])
                            se_ps = eps.tile([128, 2 * M_HALF, 1], FP32, tag="se")
                            for cc in range(2 * M_HALF):
                                nc.tensor.matmul(out=se_ps[:, cc, :],
                                                 lhsT=pnf_sb[:, cc, :],
                                                 rhs=rs_sb[:, 0:1],
                                                 start=True, stop=True)
                            a_sb = eout.tile([128, 2 * M_HALF], FP32, tag="a")
                            nc.vector.tensor_mul(out=a_sb[:],
                                                 in0=p_sb[:, :, 0],
                                                 in1=se_ps[:, :, 0])
                            nc.sync.dma_start(
                                out=alpha_o[b:b + 1, :].rearrange(
                                    "o (c p) -> p (o c)", p=128),
                                in_=a_sb[:])

            for l in range(nlayers):
                feat_phase(l)
                allgather(l)
                edge_phase(l)

    nc.compile()
    return nc


# ----------------------------------------------------------------------------
# Entry point
# ----------------------------------------------------------------------------

def make_in_maps(X, plan, wexts):
    in_maps = []
    for k in range(NCORES):
        x0T = np.zeros((XCOLS0, NROWPAD), BF16)
        x0T[:200, :NSHARD] = np.asarray(
            X[k * NSHARD:(k + 1) * NSHARD], BF16).T
        cnt = np.zeros((1, 128), np.int32)
        cnt[0] = plan[k]["CNT"]
        m = dict(
            x0T=x0T,
            PT=plan[k]["PT"], PN=plan[k]["PN"],
            IDX=plan[k]["IDX"], CNT=cnt,
        )
        for l in range(5):
            m[f"wext{l}"] = wexts[l]
        in_maps.append(m)
    return in_maps


def prep_weights(params):
    wexts = []
    for l in range(5):
        W = np.asarray(params[f"W{l}"], np.float32)
        a_s = np.asarray(params[f"as{l}"], np.float32)
        a_d = np.asarray(params[f"ad{l}"], np.float32)
        bias = np.asarray(params[f"b{l}"], np.float32)
        assert np.abs(bias).max() == 0.0, "nonzero bias not implemented"
        fin_pad = XCOLS0 if l == 0 else COLS
        cols = COLS4 if l == 4 else COLS
        wexts.append(build_wext(W, a_s, a_d, HEADS[l], fin_pad, cols))
    return wexts


_CACHE = {}


def kernel(X, edge_index, **params):
    global LAST_RESULT
    from concourse.bass_utils import run_bass_kernel_spmd

    nlayers = int(os.environ.get("GAT_NLAYERS", "5"))

    X = np.asarray(X, np.float32)
    edge_index = np.asarray(edge_index)
    in_dtype = edge_index.dtype

    plan, (ac, ab, aslot) = build_plan(edge_index)
    wexts = prep_weights(params)

    if "prog" not in _CACHE or _CACHE.get("nlayers") != nlayers:
        _CACHE["prog"] = build_program(nlayers)
        _CACHE["nlayers"] = nlayers
    nc = _CACHE["prog"]

    in_maps = make_in_maps(X, plan, wexts)

    trace = os.environ.get("GAT_TRACE", "0") == "1"
    res = run_bass_kernel_spmd(nc, in_maps, core_ids=list(range(NCORES)),
                               trace=trace)
    LAST_RESULT = res

    x = np.concatenate([res.results[k]["out4"] for k in range(NCORES)], axis=0)
    alpha_store = np.stack([res.results[k]["alpha"] for k in range(NCORES)])
    alpha = alpha_store[ac, ab, aslot].astype(np.float32).reshape(-1, 1)
    return np.asarray(x, np.float32), alpha
